# revision 5
# baseline (speedup 1.0000x reference)
"""DoubleGRU Trainium2 Bass kernel.

Strategy
--------
Data-parallel over batch across 8 NeuronCores. All activations live in
[feature=128 partitions, batch (free dim)] layout on-device so every matmul
streams activation columns through a stationary [128,128] weight (out =
W.T @ actT = (act @ W).T) with zero on-device transposes; the layout change
happens on the host (numpy transpose + bf16 cast) before upload / after
download.

The kernel is ScalarE(ACT)-bound: the 6 transcendental gates are the
irreducible work. So: x/old_h/out move as bf16 (host casts), the oh->bf16
copy and the relu are kept OFF the ACT engine, the two sigmoids of each
stage run as ONE [128,1024] activation call, and all matmuls run bf16
(1 PE cycle/row) split between VectorE / GpSimd so neither passes ACT.
DMA runs in 1 MiB-ish transfers (4 compute chunks per transfer) on the
HWDGE sync ring. A 2-stage software pipeline (stage-1 of chunk c+1 emitted
before stage-2 of chunk c) keeps every engine queue fed.
"""
import os
import sys

sys.path.insert(0, "/opt/trn_rl_repo")

import numpy as np

import concourse.bass as bass
import concourse.tile as tile
from concourse import bacc, bass_utils, mybir

B = 131072
S = 128
NCORES = 8
BC = B // NCORES  # 16384 batch rows per core
CH = 512          # batch columns per compute chunk
NCH = BC // CH
GRP = int(os.environ.get("K_GRP", "4"))  # compute chunks per DMA transfer
DCH = CH * GRP

F32 = mybir.dt.float32
BF16 = mybir.dt.bfloat16
F8 = mybir.dt.float8e4
AF = mybir.ActivationFunctionType
ALU = mybir.AluOpType

_NC_CACHE = {}

BUFS_ACTS = int(os.environ.get("K_BUFS_ACTS", "5"))
BUFS_INP = int(os.environ.get("K_BUFS_INP", "4"))
BUFS_OUT = int(os.environ.get("K_BUFS_OUT", "4"))
VARIANT = os.environ.get("K_VARIANT", "full")
# engine dials for the four gating muls / relu
S2_ENG = os.environ.get("K_S2", "gps")      # z * (oh - ht)
S4_ENG = os.environ.get("K_S4", "gps")      # z2 * (mid_h - ht2)
RELU_ENG = os.environ.get("K_RELU", "dve")  # relu(ps_mid) -> bf16
MERGE_ZR = os.environ.get("K_MERGE_ZR", "1") == "1"
ODMA = os.environ.get("K_ODMA", "act")      # engine issuing the output DMA
# fp8 DoubleRow pairs for the z/r and z2/r2 gates ("", "zr", "zr+z2r2")
FP8 = os.environ.get("K_F8", "")


def _eng(nc, name):
    return {"gps": nc.gpsimd, "dve": nc.vector, "act": nc.scalar}[name]


def _build(use_bias: bool, repeat: int = 1, compile: bool = True):
    nc = bacc.Bacc("TRN2", target_bir_lowering=False, debug=False, num_devices=NCORES)

    f8_zr = "zr" in FP8
    f8_z2 = "z2r2" in FP8

    xT = nc.dram_tensor("xT", [S, BC], BF16, kind="ExternalInput").ap()
    ohT = nc.dram_tensor("ohT", [S, BC], BF16, kind="ExternalInput").ap()
    wx1 = nc.dram_tensor("wx1", [3, S, S], BF16, kind="ExternalInput").ap()
    wx2 = nc.dram_tensor("wx2", [3, S, S], BF16, kind="ExternalInput").ap()
    wh = nc.dram_tensor("wh", [6, S, S], BF16, kind="ExternalInput").ap()
    bT = nc.dram_tensor("bT", [6, S, 1], F32, kind="ExternalInput").ap()
    midw = nc.dram_tensor("midw", [S, S], BF16, kind="ExternalInput").ap()
    outT = nc.dram_tensor("outT", [S, BC], BF16, kind="ExternalOutput").ap()
    if f8_zr:
        # fused fp8 input pairs + fp8 weight pairs, host-packed
        xoh8 = nc.dram_tensor("xoh8", [S, 2, BC], F8, kind="ExternalInput").ap()
        wp8 = nc.dram_tensor("wp8", [2, S, 2, S], F8, kind="ExternalInput").ap()
    if f8_z2:
        w28 = nc.dram_tensor("w28", [2, S, 2, S], F8, kind="ExternalInput").ap()

    with tile.TileContext(nc) as tc:
        with (
            tc.tile_pool(name="wpool", bufs=1) as wp,
            tc.tile_pool(name="inp", bufs=BUFS_INP) as inp,
            tc.tile_pool(name="acts", bufs=BUFS_ACTS) as acts,
            tc.tile_pool(name="outp", bufs=BUFS_OUT) as outp,
            tc.tile_pool(name="ps", bufs=1, space="PSUM") as psp,
            tc.tile_pool(name="ps2", bufs=2, space="PSUM") as psp2,
        ):
            def load_w(src, tag):
                t = wp.tile([S, S], BF16, tag=tag)
                nc.sync.dma_start(out=t, in_=src)
                return t

            w_x1 = [load_w(wx1[i, :, :], f"wx1{i}") for i in range(3)]
            w_x2 = [load_w(wx2[i, :, :], f"wx2{i}") for i in range(3)]
            w_h = [load_w(wh[i, :, :], f"wh{i}") for i in range(6)]
            w_mid = load_w(midw, "wmid")
            if f8_zr:
                wzp = wp.tile([S, 2, S], F8, tag="wzp")
                nc.sync.dma_start(out=wzp, in_=wp8[0])
                wrp = wp.tile([S, 2, S], F8, tag="wrp")
                nc.sync.dma_start(out=wrp, in_=wp8[1])
            if f8_z2:
                wz2p = wp.tile([S, 2, S], F8, tag="wz2p")
                nc.sync.dma_start(out=wz2p, in_=w28[0])
                wr2p = wp.tile([S, 2, S], F8, tag="wr2p")
                nc.sync.dma_start(out=wr2p, in_=w28[1])

            b_t = []
            if use_bias:
                for i in range(6):
                    t = wp.tile([S, 1], F32, tag=f"b{i}")
                    nc.sync.dma_start(out=t, in_=bT[i, :, :])
                    b_t.append(t)

            def act(out_ap, in_ap, func, bias_idx):
                if use_bias:
                    nc.scalar.activation(out_ap, in_ap, func, bias=b_t[bias_idx][:])
                else:
                    nc.scalar.activation(out_ap, in_ap, func)

            if VARIANT == "dma":
                for it in range(NCH * repeat):
                    c = it % NCH
                    g = c % GRP
                    if g == 0:
                        gs = bass.ts(c // GRP, DCH)
                        x4 = inp.tile([S, DCH], BF16, tag="x")
                        nc.sync.dma_start(out=x4, in_=xT[:, gs])
                        oh4 = inp.tile([S, DCH], BF16, tag="oh")
                        nc.sync.dma_start(out=oh4, in_=ohT[:, gs])
                    if g == GRP - 1:
                        nc.sync.dma_start(out=outT[:, gs], in_=x4[:])

            if VARIANT == "mm":
                xs = wp.tile([S, CH], BF16, tag="xs")
                nc.vector.memset(xs, 0.25)
                ohs = wp.tile([S, CH], BF16, tag="ohs")
                nc.vector.memset(ohs, 0.25)
                for it in range(NCH * repeat):
                    ps_zr = psp.tile([S, 2 * CH], F32, tag="ps_zr")
                    nc.tensor.matmul(ps_zr[:, :CH], w_x1[0][:], xs[:], start=True, stop=False)
                    nc.tensor.matmul(ps_zr[:, :CH], w_h[0][:], ohs[:], start=False, stop=True)
                    nc.tensor.matmul(ps_zr[:, CH:], w_x1[1][:], xs[:], start=True, stop=False)
                    nc.tensor.matmul(ps_zr[:, CH:], w_h[1][:], ohs[:], start=False, stop=True)
                    ps_ht = psp2.tile([S, CH], F32, tag="ps_ht")
                    nc.tensor.matmul(ps_ht[:], w_x1[2][:], xs[:], start=True, stop=False)
                    nc.tensor.matmul(ps_ht[:], w_h[2][:], ohs[:], start=False, stop=True)
                    ps_mid = psp.tile([S, CH], F32, tag="ps_mid")
                    nc.tensor.matmul(ps_mid[:], w_mid[:], ohs[:], start=True, stop=True)
                    ps_zr2 = psp.tile([S, 2 * CH], F32, tag="ps_zr2")
                    nc.tensor.matmul(ps_zr2[:, :CH], w_x2[0][:], xs[:], start=True, stop=False)
                    nc.tensor.matmul(ps_zr2[:, :CH], w_h[0][:], ohs[:], start=False, stop=True)
                    nc.tensor.matmul(ps_zr2[:, CH:], w_x2[1][:], xs[:], start=True, stop=False)
                    nc.tensor.matmul(ps_zr2[:, CH:], w_h[4][:], ohs[:], start=False, stop=True)
                    ps_ht2 = psp.tile([S, CH], F32, tag="ps_ht2")
                    nc.tensor.matmul(ps_ht2[:], w_x2[2][:], xs[:], start=True, stop=False)
                    nc.tensor.matmul(ps_ht2[:], w_h[5][:], ohs[:], start=False, stop=True)
                hf = outp.tile([S, DCH], BF16, tag="h")
                nc.vector.memset(hf, 0.0)
                nc.sync.dma_start(out=outT[:, 0:DCH], in_=hf[:])

            if VARIANT == "full":
                group_state = [None]

                def emit_s1(it):
                    c = it % NCH
                    g = c % GRP
                    if g == 0:
                        gs = bass.ts(c // GRP, DCH)
                        x4 = inp.tile([S, DCH], BF16, tag="x")
                        oh4 = inp.tile([S, DCH], BF16, tag="oh")
                        if not f8_zr:
                            nc.sync.dma_start(out=x4, in_=xT[:, gs])
                        nc.sync.dma_start(out=oh4, in_=ohT[:, gs])
                        xoh4 = None
                        if f8_zr:
                            xoh4 = inp.tile([S, 2, DCH], F8, tag="xoh")
                            nc.sync.dma_start(out=xoh4, in_=xoh8[:, :, gs])
                            nc.sync.dma_start(out=x4, in_=xT[:, gs])
                        h4 = outp.tile([S, DCH], BF16, tag="h")
                        group_state[0] = (gs, x4, oh4, xoh4, h4)
                    gs, x4, oh4, xoh4, h4 = group_state[0]
                    sl = bass.ts(g, CH)
                    x_t, oh_t = x4[:, sl], oh4[:, sl]

                    ps_zr = psp.tile([S, 2 * CH], F32, tag="ps_zr")
                    if f8_zr:
                        mv = xoh4[:, 0:2, sl]
                        nc.tensor.matmul(ps_zr[:, CH:], wrp[:, 0:2, :], mv,
                                         start=True, stop=True,
                                         perf_mode=mybir.MatmulPerfMode.DoubleRow)
                        nc.tensor.matmul(ps_zr[:, :CH], wzp[:, 0:2, :], mv,
                                         start=True, stop=True,
                                         perf_mode=mybir.MatmulPerfMode.DoubleRow)
                    else:
                        nc.tensor.matmul(ps_zr[:, CH:], w_x1[1][:], x_t, start=True, stop=False)
                        nc.tensor.matmul(ps_zr[:, CH:], w_h[1][:], oh_t, start=False, stop=True)
                        nc.tensor.matmul(ps_zr[:, :CH], w_x1[0][:], x_t, start=True, stop=False)
                        nc.tensor.matmul(ps_zr[:, :CH], w_h[0][:], oh_t, start=False, stop=True)
                    zr = acts.tile([S, 2 * CH], BF16, tag="zr")
                    if MERGE_ZR and not use_bias:
                        nc.scalar.activation(zr[:], ps_zr[:], AF.Sigmoid)
                    else:
                        act(zr[:, CH:], ps_zr[:, CH:], AF.Sigmoid, 1)
                        act(zr[:, :CH], ps_zr[:, :CH], AF.Sigmoid, 0)
                    z, r = zr[:, :CH], zr[:, CH:]

                    rh = acts.tile([S, CH], BF16, tag="rh")
                    nc.vector.tensor_mul(rh[:], r, oh_t)
                    ps_ht = psp2.tile([S, CH], F32, tag="ps_ht")
                    nc.tensor.matmul(ps_ht[:], w_x1[2][:], x_t, start=True, stop=False)
                    nc.tensor.matmul(ps_ht[:], w_h[2][:], rh[:], start=False, stop=True)
                    ht = acts.tile([S, CH], BF16, tag="ht")
                    act(ht[:], ps_ht[:], AF.Tanh, 2)

                    s1 = acts.tile([S, CH], BF16, tag="s1")
                    nc.vector.tensor_sub(s1[:], oh_t, ht[:])
                    s2 = acts.tile([S, CH], BF16, tag="s2")
                    _eng(nc, S2_ENG).tensor_mul(s2[:], z, s1[:])
                    mid_h = acts.tile([S, CH], BF16, tag="mid_h")
                    nc.vector.tensor_add(mid_h[:], ht[:], s2[:])
                    return dict(gs=gs, g=g, sl=sl, h4=h4, mid_h=mid_h)

                def emit_s2(st):
                    gs, g, sl, h4, mid_h = st["gs"], st["g"], st["sl"], st["h4"], st["mid_h"]

                    ps_mid = psp.tile([S, CH], F32, tag="ps_mid")
                    nc.tensor.matmul(ps_mid[:], w_mid[:], mid_h[:], start=True, stop=True)
                    mid_x = acts.tile([S, CH], BF16, tag="mid_x")
                    if RELU_ENG == "act":
                        nc.scalar.activation(mid_x[:], ps_mid[:], AF.Relu)
                    else:
                        _eng(nc, RELU_ENG).tensor_scalar_max(mid_x[:], ps_mid[:], 0.0)

                    ps_zr2 = psp.tile([S, 2 * CH], F32, tag="ps_zr2")
                    nc.tensor.matmul(ps_zr2[:, CH:], w_x2[1][:], mid_x[:], start=True, stop=False)
                    nc.tensor.matmul(ps_zr2[:, CH:], w_h[4][:], mid_h[:], start=False, stop=True)
                    nc.tensor.matmul(ps_zr2[:, :CH], w_x2[0][:], mid_x[:], start=True, stop=False)
                    nc.tensor.matmul(ps_zr2[:, :CH], w_h[0][:], mid_h[:], start=False, stop=True)
                    zr2 = acts.tile([S, 2 * CH], BF16, tag="zr2")
                    if MERGE_ZR and not use_bias:
                        nc.scalar.activation(zr2[:], ps_zr2[:], AF.Sigmoid)
                    else:
                        act(zr2[:, CH:], ps_zr2[:, CH:], AF.Sigmoid, 4)
                        act(zr2[:, :CH], ps_zr2[:, :CH], AF.Sigmoid, 0)
                    z2, r2 = zr2[:, :CH], zr2[:, CH:]

                    r2h = acts.tile([S, CH], BF16, tag="r2h")
                    nc.vector.tensor_mul(r2h[:], r2, mid_h[:])
                    ps_ht2 = psp.tile([S, CH], F32, tag="ps_ht2")
                    nc.tensor.matmul(ps_ht2[:], w_x2[2][:], mid_x[:], start=True, stop=False)
                    nc.tensor.matmul(ps_ht2[:], w_h[5][:], r2h[:], start=False, stop=True)
                    ht2 = acts.tile([S, CH], BF16, tag="ht2")
                    act(ht2[:], ps_ht2[:], AF.Tanh, 5)

                    s3 = acts.tile([S, CH], BF16, tag="s3")
                    nc.vector.tensor_sub(s3[:], mid_h[:], ht2[:])
                    s4 = acts.tile([S, CH], BF16, tag="s4")
                    _eng(nc, S4_ENG).tensor_mul(s4[:], z2, s3[:])
                    nc.vector.tensor_add(h4[:, sl], ht2[:], s4[:])
                    if g == GRP - 1:
                        eng = {"act": nc.scalar, "gps": nc.gpsimd,
                               "sync": nc.sync}[ODMA]
                        eng.dma_start(out=outT[:, gs], in_=h4[:])

                n_it = NCH * repeat
                pending = emit_s1(0)
                for it in range(n_it):
                    nxt = emit_s1(it + 1) if it + 1 < n_it else None
                    emit_s2(pending)
                    pending = nxt

    if compile:
        nc.compile()
    return nc


def _to_bf16(a):
    import ml_dtypes
    return np.ascontiguousarray(a.astype(ml_dtypes.bfloat16))


def kernel(x, old_h, W_x1, W_x2, W_h, b, mid, trace=False):
    import ml_dtypes

    x = np.asarray(x, dtype=np.float32)
    old_h = np.asarray(old_h, dtype=np.float32)
    W_x1 = np.ascontiguousarray(W_x1, dtype=np.float32)
    W_x2 = np.ascontiguousarray(W_x2, dtype=np.float32)
    W_h = np.ascontiguousarray(W_h, dtype=np.float32)
    b = np.asarray(b, dtype=np.float32)
    mid = np.ascontiguousarray(mid, dtype=np.float32)
    assert x.shape == (B, S) and old_h.shape == (B, S)

    use_bias = bool(np.any(b != 0.0))
    key = use_bias
    if key not in _NC_CACHE:
        _NC_CACHE[key] = _build(use_bias)
    nc = _NC_CACHE[key]

    xT = _to_bf16(x.T)      # [S, B]
    ohT = _to_bf16(old_h.T)
    bTh = np.ascontiguousarray(b.reshape(6, 1, S).transpose(0, 2, 1))  # [6,S,1]
    wx1_b = _to_bf16(W_x1)
    wx2_b = _to_bf16(W_x2)
    wh_b = _to_bf16(W_h)
    mid_b = _to_bf16(mid)

    f8_zr = "zr" in FP8
    f8 = ml_dtypes.float8_e4m3
    if f8_zr:
        xT8 = np.ascontiguousarray(x.T).astype(f8)
        ohT8 = np.ascontiguousarray(old_h.T).astype(f8)
        # weight pairs [2, S(k), 2(slab), S(m)]: slab0 = W_x1[i], slab1 = W_h[i]
        wp8 = np.stack([
            np.stack([W_x1[0], W_h[0]], axis=1),
            np.stack([W_x1[1], W_h[1]], axis=1),
        ]).astype(f8)
        wp8 = np.ascontiguousarray(wp8)

    in_maps = []
    for c in range(NCORES):
        sl = slice(c * BC, (c + 1) * BC)
        m = {
            "xT": np.ascontiguousarray(xT[:, sl]),
            "ohT": np.ascontiguousarray(ohT[:, sl]),
            "wx1": wx1_b,
            "wx2": wx2_b,
            "wh": wh_b,
            "bT": bTh,
            "midw": mid_b,
        }
        if f8_zr:
            m["xoh8"] = np.ascontiguousarray(
                np.stack([xT8[:, sl], ohT8[:, sl]], axis=1))
            m["wp8"] = wp8
        in_maps.append(m)

    res = bass_utils.run_bass_kernel_spmd(
        nc, in_maps, core_ids=list(range(NCORES)), trace=trace
    )
    outT = np.concatenate(
        [np.asarray(res.results[c]["outT"]) for c in range(NCORES)], axis=1)
    h = np.ascontiguousarray(outT.T.astype(np.float32))
    if trace:
        return (h,), res
    return (h,)


# revision 11
# speedup vs baseline: 1.2826x; 1.2826x over previous
"""DoubleGRU Trainium2 Bass kernel.

Strategy
--------
Data-parallel over batch across 8 NeuronCores. All activations live in
[feature=128 partitions, batch (free dim)] layout on-device so every matmul
streams activation columns through a stationary [128,128] weight (out =
W.T @ actT = (act @ W).T) with zero on-device transposes; the layout change
happens on the host (numpy transpose) before upload / after download.

Matmuls run in float32r (fp32 with 11-bit mantissa, 1 PE cycle/row at free
dim >= 256 vs 4 cycles/row for plain fp32). PSUM accumulates the x-path and
h-path matmuls of each gate. Sigmoid/tanh run on ScalarE reading PSUM
directly. Elementwise GRU combines are split between VectorE (ops feeding
matmuls / reading PSUM) and GpSimd (pure-SBUF f32 ops). DMA runs in 1 MiB
transfers (4 compute chunks per transfer) on the HWDGE sync ring.
"""
import os
import sys

sys.path.insert(0, "/opt/trn_rl_repo")

import numpy as np

import concourse.bass as bass
import concourse.tile as tile
from concourse import bacc, bass_utils, mybir

B = 131072
S = 128
NCORES = 8
BC = B // NCORES  # 16384 batch rows per core
CH = 512          # batch columns per compute chunk (max fp32 matmul free dim)
NCH = BC // CH
GRP = int(os.environ.get("K_GRP", "4"))  # compute chunks per DMA transfer
DCH = CH * GRP

F32R = mybir.dt.float32r
F32 = mybir.dt.float32
BF16 = mybir.dt.bfloat16
AF = mybir.ActivationFunctionType

_NC_CACHE = {}


BUFS_ACTS = int(os.environ.get("K_BUFS_ACTS", "5"))
BUFS_INP = int(os.environ.get("K_BUFS_INP", "4"))
BUFS_OUT = int(os.environ.get("K_BUFS_OUT", "4"))
CHAIN_V2 = os.environ.get("K_CHAIN_V2", "1") == "1"
VARIANT = os.environ.get("K_VARIANT", "full")
SWDGE_X = os.environ.get("K_SWDGE_X", "0") == "1"


def _build(use_bias: bool, repeat: int = 1, compile: bool = True):
    nc = bacc.Bacc("TRN2", target_bir_lowering=False, debug=False, num_devices=NCORES)

    xT = nc.dram_tensor("xT", [S, BC], F32R, kind="ExternalInput").ap()
    ohT = nc.dram_tensor("ohT", [S, BC], F32R, kind="ExternalInput").ap()
    wx1 = nc.dram_tensor("wx1", [3, S, S], F32R, kind="ExternalInput").ap()
    wx2 = nc.dram_tensor("wx2", [3, S, S], F32R, kind="ExternalInput").ap()
    wh = nc.dram_tensor("wh", [6, S, S], F32R, kind="ExternalInput").ap()
    bT = nc.dram_tensor("bT", [6, S, 1], F32, kind="ExternalInput").ap()
    midw = nc.dram_tensor("midw", [S, S], F32R, kind="ExternalInput").ap()
    outT = nc.dram_tensor("outT", [S, BC], F32, kind="ExternalOutput").ap()

    f32 = lambda ap: ap.bitcast(F32)

    with tile.TileContext(nc) as tc:
        with (
            tc.tile_pool(name="wpool", bufs=1) as wp,
            tc.tile_pool(name="inp", bufs=BUFS_INP) as inp,
            tc.tile_pool(name="acts", bufs=BUFS_ACTS) as acts,
            tc.tile_pool(name="outp", bufs=BUFS_OUT) as outp,
            tc.tile_pool(name="ps", bufs=1, space="PSUM") as psp,
            tc.tile_pool(name="ps2", bufs=2, space="PSUM") as psp2,
        ):
            def load_w(src, tag):
                t = wp.tile([S, S], F32R, tag=tag)
                nc.sync.dma_start(out=t, in_=src)
                return t

            w_x1 = [load_w(wx1[i, :, :], f"wx1{i}") for i in range(3)]
            w_x2 = [load_w(wx2[i, :, :], f"wx2{i}") for i in range(3)]
            w_h = [load_w(wh[i, :, :], f"wh{i}") for i in range(6)]
            w_mid = load_w(midw, "wmid")

            def bcast(src, tag):
                t = wp.tile([S, S], BF16, tag=tag)
                nc.vector.tensor_copy(t[:], src[:].bitcast(F32))
                return t

            w_h2_b = bcast(w_h[2], "wh2b")
            w_h0_b = bcast(w_h[0], "wh0b")
            w_h4_b = bcast(w_h[4], "wh4b")
            w_h5_b = bcast(w_h[5], "wh5b")
            w_mid_b = bcast(w_mid, "wmidb")
            w_x2_b = [bcast(w_x2[i], f"wx2b{i}") for i in range(3)]
            b_t = []
            if use_bias:
                for i in range(6):
                    t = wp.tile([S, 1], F32, tag=f"b{i}")
                    nc.sync.dma_start(out=t, in_=bT[i, :, :])
                    b_t.append(t)

            def act(out_ap, in_ap, func, bias_idx):
                if use_bias:
                    nc.scalar.activation(out_ap, in_ap, func, bias=b_t[bias_idx][:])
                else:
                    nc.scalar.activation(out_ap, in_ap, func)

            if VARIANT == "elem2":
                xs0 = wp.tile([S, CH], F32, tag="xs0")
                nc.vector.memset(xs0, 0.25)
                for it in range(NCH * repeat):
                    for j in range(9):
                        tt = acts.tile([S, CH], F32, tag=f"e{j}")
                        nc.vector.tensor_mul(tt[:], xs0[:], xs0[:])
                hf = outp.tile([S, DCH], F32, tag="h")
                nc.vector.memset(hf, 0.0)
                nc.sync.dma_start(out=outT[:, 0:DCH], in_=hf[:])
            if VARIANT == "elem3":
                xs0 = wp.tile([S, CH], mybir.dt.bfloat16, tag="xs0")
                nc.vector.memset(xs0, 0.25)
                for it in range(NCH * repeat):
                    for j in range(9):
                        tt = acts.tile([S, CH], mybir.dt.bfloat16, tag=f"e{j}")
                        nc.vector.tensor_mul(tt[:], xs0[:], xs0[:])
                hf = outp.tile([S, DCH], F32, tag="h")
                nc.vector.memset(hf, 0.0)
                nc.sync.dma_start(out=outT[:, 0:DCH], in_=hf[:])
            if VARIANT in ("act6b", "act6f", "act6p"):
                xs0 = wp.tile([S, CH], F32, tag="xs0")
                nc.vector.memset(xs0, 0.25)
                ps0 = psp.tile([S, CH], F32, tag="ps0")
                nc.vector.memset(ps0, 0.25)
                odt = F32 if VARIANT == "act6f" else BF16
                src = ps0 if VARIANT == "act6p" else xs0
                for it in range(NCH * repeat):
                    for j in range(6):
                        t = acts.tile([S, CH], odt, tag=f"a{j}")
                        nc.scalar.activation(t[:], src[:], AF.Sigmoid)
                hf = outp.tile([S, DCH], F32, tag="h")
                nc.vector.memset(hf, 0.0)
                nc.sync.dma_start(out=outT[:, 0:DCH], in_=hf[:])
            if VARIANT == "vec7":
                xs0 = wp.tile([S, CH], BF16, tag="xs0")
                nc.vector.memset(xs0, 0.25)
                for it in range(NCH * repeat):
                    for j in range(7):
                        t = acts.tile([S, CH], BF16, tag=f"v{j}")
                        nc.vector.tensor_mul(t[:], xs0[:], xs0[:])
                hf = outp.tile([S, DCH], F32, tag="h")
                nc.vector.memset(hf, 0.0)
                nc.sync.dma_start(out=outT[:, 0:DCH], in_=hf[:])
            if VARIANT == "elem":
                xs0 = wp.tile([S, CH], F32, tag="xs0")
                nc.vector.memset(xs0, 0.25)
                for it in range(NCH * repeat):
                    z = acts.tile([S, CH], F32, tag="z")
                    nc.vector.tensor_mul(z[:], xs0[:], xs0[:])       # rh
                    m1 = acts.tile([S, CH], F32, tag="t1")
                    nc.gpsimd.tensor_mul(m1[:], xs0[:], xs0[:])
                    zc = acts.tile([S, CH], F32, tag="zc")
                    nc.gpsimd.tensor_scalar(zc[:], xs0[:], -1.0, 1.0,
                                            mybir.AluOpType.mult, mybir.AluOpType.add)
                    m2 = acts.tile([S, CH], F32, tag="t2")
                    nc.gpsimd.tensor_mul(m2[:], zc[:], m1[:])
                    mid_h = acts.tile([S, CH], F32R, tag="mid_h")
                    nc.vector.tensor_add(mid_h[:], m1[:], m2[:])
                    mid_x = acts.tile([S, CH], F32R, tag="mid_x")
                    nc.vector.tensor_scalar_max(mid_x[:], z[:], 0.0)  # sbuf relu stand-in
                    r2h = acts.tile([S, CH], F32R, tag="r2h")
                    nc.vector.tensor_mul(r2h[:], z[:], f32(mid_h[:]))
                    t3 = acts.tile([S, CH], F32, tag="t3")
                    nc.gpsimd.tensor_sub(t3[:], f32(mid_h[:]), m2[:])
                    t4 = acts.tile([S, CH], F32, tag="t4")
                    nc.gpsimd.tensor_mul(t4[:], z[:], t3[:])
                    h4e = acts.tile([S, CH], F32, tag="h4e")
                    nc.gpsimd.tensor_add(h4e[:], m2[:], t4[:])
                hf = outp.tile([S, DCH], F32, tag="h")
                nc.vector.memset(hf, 0.0)
                nc.sync.dma_start(out=outT[:, 0:DCH], in_=hf[:])
            if VARIANT in ("mm", "mmact"):
                xs0 = wp.tile([S, CH], F32, tag="xs0")
                nc.vector.memset(xs0, 0.25)
                xs = wp.tile([S, CH], F32R, tag="xs")
                nc.vector.tensor_copy(xs[:], xs0[:])
                ohs = wp.tile([S, CH], F32R, tag="ohs")
                nc.vector.tensor_copy(ohs[:], xs0[:])
                for it in range(NCH * repeat):
                    ps_zr = psp.tile([S, 2 * CH], F32, tag="ps_zr")
                    nc.tensor.matmul(ps_zr[:, :CH], w_x1[0][:], xs[:], start=True, stop=False)
                    nc.tensor.matmul(ps_zr[:, :CH], w_h[0][:], ohs[:], start=False, stop=True)
                    nc.tensor.matmul(ps_zr[:, CH:], w_x1[1][:], xs[:], start=True, stop=False)
                    nc.tensor.matmul(ps_zr[:, CH:], w_h[1][:], ohs[:], start=False, stop=True)
                    ps_ht = psp2.tile([S, CH], F32, tag="ps_ht")
                    nc.tensor.matmul(ps_ht[:], w_x1[2][:], xs[:], start=True, stop=False)
                    nc.tensor.matmul(ps_ht[:], w_h[2][:], ohs[:], start=False, stop=True)
                    ps_mid = psp.tile([S, CH], F32, tag="ps_mid")
                    nc.tensor.matmul(ps_mid[:], w_mid[:], ohs[:], start=True, stop=True)
                    ps_zr2 = psp.tile([S, 2 * CH], F32, tag="ps_zr2")
                    nc.tensor.matmul(ps_zr2[:, :CH], w_x2[0][:], xs[:], start=True, stop=False)
                    nc.tensor.matmul(ps_zr2[:, :CH], w_h[0][:], ohs[:], start=False, stop=True)
                    nc.tensor.matmul(ps_zr2[:, CH:], w_x2[1][:], xs[:], start=True, stop=False)
                    nc.tensor.matmul(ps_zr2[:, CH:], w_h[4][:], ohs[:], start=False, stop=True)
                    ps_ht2 = psp.tile([S, CH], F32, tag="ps_ht2")
                    nc.tensor.matmul(ps_ht2[:], w_x2[2][:], xs[:], start=True, stop=False)
                    nc.tensor.matmul(ps_ht2[:], w_h[5][:], ohs[:], start=False, stop=True)
                    if VARIANT == "mmact":
                        zr = acts.tile([S, 2 * CH], F32, tag="zr")
                        nc.scalar.activation(zr[:], ps_zr[:], AF.Sigmoid)
                        ht = acts.tile([S, CH], F32, tag="ht")
                        nc.scalar.activation(ht[:], ps_ht[:], AF.Tanh)
                        mx = acts.tile([S, CH], F32, tag="mid_x")
                        nc.vector.tensor_scalar_max(mx[:], ps_mid[:], 0.0)
                        zr2 = acts.tile([S, 2 * CH], F32, tag="zr2")
                        nc.scalar.activation(zr2[:], ps_zr2[:], AF.Sigmoid)
                        ht2 = acts.tile([S, CH], F32, tag="ht2")
                        nc.scalar.activation(ht2[:], ps_ht2[:], AF.Tanh)
                hf = outp.tile([S, DCH], F32, tag="h")
                nc.vector.memset(hf, 0.0)
                nc.sync.dma_start(out=outT[:, 0:DCH], in_=hf[:])
            for it in range(NCH * repeat if VARIANT in ("dma", "dmamm") else 0):
                c = it % NCH
                g = c % GRP
                if g == 0:
                    gs = bass.ts(c // GRP, DCH)
                    x4 = inp.tile([S, DCH], F32R, tag="x")
                    (nc.gpsimd if SWDGE_X else nc.sync).dma_start(out=x4, in_=xT[:, gs])
                    oh4 = inp.tile([S, DCH], F32R, tag="oh")
                    nc.sync.dma_start(out=oh4, in_=ohT[:, gs])
                    h4 = outp.tile([S, DCH], F32, tag="h")
                sl = bass.ts(g, CH)
                x_t = x4[:, sl]
                oh_t = oh4[:, sl]

                if VARIANT == "dma":
                    if g == GRP - 1:
                        nc.sync.dma_start(out=outT[:, gs], in_=f32(x4[:]))
                    continue
                if VARIANT == "dmamm":
                    ps_zr = psp.tile([S, 2 * CH], F32, tag="ps_zr")
                    nc.tensor.matmul(ps_zr[:, :CH], w_x1[0][:], x_t, start=True, stop=False)
                    nc.tensor.matmul(ps_zr[:, :CH], w_h[0][:], oh_t, start=False, stop=True)
                    nc.tensor.matmul(ps_zr[:, CH:], w_x1[1][:], x_t, start=True, stop=False)
                    nc.tensor.matmul(ps_zr[:, CH:], w_h[1][:], oh_t, start=False, stop=True)
                    ps_ht = psp2.tile([S, CH], F32, tag="ps_ht")
                    nc.tensor.matmul(ps_ht[:], w_x1[2][:], x_t, start=True, stop=False)
                    nc.tensor.matmul(ps_ht[:], w_h[2][:], oh_t, start=False, stop=True)
                    ps_mid = psp.tile([S, CH], F32, tag="ps_mid")
                    nc.tensor.matmul(ps_mid[:], w_mid[:], oh_t, start=True, stop=True)
                    ps_zr2 = psp.tile([S, 2 * CH], F32, tag="ps_zr2")
                    nc.tensor.matmul(ps_zr2[:, :CH], w_x2[0][:], x_t, start=True, stop=False)
                    nc.tensor.matmul(ps_zr2[:, :CH], w_h[0][:], oh_t, start=False, stop=True)
                    nc.tensor.matmul(ps_zr2[:, CH:], w_x2[1][:], x_t, start=True, stop=False)
                    nc.tensor.matmul(ps_zr2[:, CH:], w_h[4][:], oh_t, start=False, stop=True)
                    ps_ht2 = psp.tile([S, CH], F32, tag="ps_ht2")
                    nc.tensor.matmul(ps_ht2[:], w_x2[2][:], x_t, start=True, stop=False)
                    nc.tensor.matmul(ps_ht2[:], w_h[5][:], oh_t, start=False, stop=True)
                    if g == GRP - 1:
                        nc.sync.dma_start(out=outT[:, gs], in_=f32(x4[:]))
                    continue



            if VARIANT == "full":
                # explicit 2-stage software pipeline: stage-1 of chunk c+1 is
                # emitted BEFORE stage-2 of chunk c so each in-order engine
                # stream alternates independent work and PE gaps get filled.
                group_state = [None]

                def emit_s1(it):
                    c = it % NCH
                    g = c % GRP
                    if g == 0:
                        gs = bass.ts(c // GRP, DCH)
                        x4 = inp.tile([S, DCH], F32R, tag="x")
                        (nc.gpsimd if SWDGE_X else nc.sync).dma_start(out=x4, in_=xT[:, gs])
                        oh4 = inp.tile([S, DCH], F32R, tag="oh")
                        nc.sync.dma_start(out=oh4, in_=ohT[:, gs])
                        h4 = outp.tile([S, DCH], F32, tag="h")
                        group_state[0] = (gs, x4, oh4, h4)
                    gs, x4, oh4, h4 = group_state[0]
                    sl = bass.ts(g, CH)
                    x_t, oh_t = x4[:, sl], oh4[:, sl]

                    oh_b = acts.tile([S, CH], BF16, tag="ohb")
                    nc.scalar.activation(oh_b[:], f32(oh_t), AF.Copy)

                    ps_zr = psp.tile([S, 2 * CH], F32, tag="ps_zr")
                    nc.tensor.matmul(ps_zr[:, CH:], w_x1[1][:], x_t, start=True, stop=False)
                    nc.tensor.matmul(ps_zr[:, CH:], w_h[1][:], oh_t, start=False, stop=True)
                    zr = acts.tile([S, 2 * CH], BF16, tag="zr")
                    # r first: it gates the rh -> h_tilde chain
                    act(zr[:, CH:], ps_zr[:, CH:], AF.Sigmoid, 1)
                    nc.tensor.matmul(ps_zr[:, :CH], w_x1[0][:], x_t, start=True, stop=False)
                    nc.tensor.matmul(ps_zr[:, :CH], w_h[0][:], oh_t, start=False, stop=True)
                    act(zr[:, :CH], ps_zr[:, :CH], AF.Sigmoid, 0)
                    z, r = zr[:, :CH], zr[:, CH:]

                    rh = acts.tile([S, CH], BF16, tag="rh")
                    nc.vector.tensor_mul(rh[:], r, oh_b[:])
                    ps_ht = psp2.tile([S, CH], F32, tag="ps_ht")
                    nc.tensor.matmul(ps_ht[:], w_x1[2][:], x_t, start=True, stop=False)
                    nc.tensor.matmul(ps_ht[:], w_h2_b[:], rh[:], start=False, stop=True)
                    ht = acts.tile([S, CH], BF16, tag="ht")
                    act(ht[:], ps_ht[:], AF.Tanh, 2)

                    zc = acts.tile([S, CH], BF16, tag="zc")
                    nc.gpsimd.tensor_scalar(zc[:], z, -1.0, 1.0,
                                            mybir.AluOpType.mult, mybir.AluOpType.add)
                    m1 = acts.tile([S, CH], BF16, tag="t1")
                    nc.gpsimd.tensor_mul(m1[:], z, oh_b[:])
                    m2 = acts.tile([S, CH], BF16, tag="t2")
                    nc.vector.tensor_mul(m2[:], zc[:], ht[:])
                    mid_h = acts.tile([S, CH], BF16, tag="mid_h")
                    nc.vector.tensor_add(mid_h[:], m1[:], m2[:])
                    return dict(gs=gs, g=g, sl=sl, h4=h4, mid_h=mid_h)

                def emit_s2(st):
                    gs, g, sl, h4, mid_h = st["gs"], st["g"], st["sl"], st["h4"], st["mid_h"]

                    ps_mid = psp.tile([S, CH], F32, tag="ps_mid")
                    nc.tensor.matmul(ps_mid[:], w_mid_b[:], mid_h[:], start=True, stop=True)
                    mid_x = acts.tile([S, CH], BF16, tag="mid_x")
                    nc.scalar.activation(mid_x[:], ps_mid[:], AF.Relu)

                    ps_zr2 = psp.tile([S, 2 * CH], F32, tag="ps_zr2")
                    nc.tensor.matmul(ps_zr2[:, CH:], w_x2_b[1][:], mid_x[:], start=True, stop=False)
                    nc.tensor.matmul(ps_zr2[:, CH:], w_h4_b[:], mid_h[:], start=False, stop=True)
                    zr2 = acts.tile([S, 2 * CH], BF16, tag="zr2")
                    act(zr2[:, CH:], ps_zr2[:, CH:], AF.Sigmoid, 4)
                    nc.tensor.matmul(ps_zr2[:, :CH], w_x2_b[0][:], mid_x[:], start=True, stop=False)
                    nc.tensor.matmul(ps_zr2[:, :CH], w_h0_b[:], mid_h[:], start=False, stop=True)
                    act(zr2[:, :CH], ps_zr2[:, :CH], AF.Sigmoid, 0)
                    z2, r2 = zr2[:, :CH], zr2[:, CH:]

                    r2h = acts.tile([S, CH], BF16, tag="r2h")
                    nc.vector.tensor_mul(r2h[:], r2, mid_h[:])
                    ps_ht2 = psp.tile([S, CH], F32, tag="ps_ht2")
                    nc.tensor.matmul(ps_ht2[:], w_x2_b[2][:], mid_x[:], start=True, stop=False)
                    nc.tensor.matmul(ps_ht2[:], w_h5_b[:], r2h[:], start=False, stop=True)
                    ht2 = acts.tile([S, CH], BF16, tag="ht2")
                    act(ht2[:], ps_ht2[:], AF.Tanh, 5)

                    t3 = acts.tile([S, CH], BF16, tag="t3")
                    nc.vector.tensor_sub(t3[:], mid_h[:], ht2[:])
                    t4 = acts.tile([S, CH], BF16, tag="t4")
                    nc.vector.tensor_mul(t4[:], z2, t3[:])
                    nc.vector.tensor_add(h4[:, sl], ht2[:], t4[:])
                    if g == GRP - 1:
                        nc.scalar.dma_start(out=outT[:, gs], in_=h4[:])

                n_it = NCH * repeat
                pending = emit_s1(0)
                for it in range(n_it):
                    nxt = emit_s1(it + 1) if it + 1 < n_it else None
                    emit_s2(pending)
                    pending = nxt

    if compile:
        nc.compile()
    return nc


def kernel(x, old_h, W_x1, W_x2, W_h, b, mid, trace=False):
    x = np.asarray(x, dtype=np.float32)
    old_h = np.asarray(old_h, dtype=np.float32)
    W_x1 = np.ascontiguousarray(W_x1, dtype=np.float32)
    W_x2 = np.ascontiguousarray(W_x2, dtype=np.float32)
    W_h = np.ascontiguousarray(W_h, dtype=np.float32)
    b = np.asarray(b, dtype=np.float32)
    mid = np.ascontiguousarray(mid, dtype=np.float32)
    assert x.shape == (B, S) and old_h.shape == (B, S)

    use_bias = bool(np.any(b != 0.0))
    key = use_bias
    if key not in _NC_CACHE:
        _NC_CACHE[key] = _build(use_bias)
    nc = _NC_CACHE[key]

    xT = np.ascontiguousarray(x.T)      # [S, B]
    ohT = np.ascontiguousarray(old_h.T)
    bTh = np.ascontiguousarray(b.reshape(6, 1, S).transpose(0, 2, 1))  # [6,S,1]

    in_maps = []
    for c in range(NCORES):
        sl = slice(c * BC, (c + 1) * BC)
        in_maps.append({
            "xT": np.ascontiguousarray(xT[:, sl]),
            "ohT": np.ascontiguousarray(ohT[:, sl]),
            "wx1": W_x1,
            "wx2": W_x2,
            "wh": W_h,
            "bT": bTh,
            "midw": mid,
        })

    res = bass_utils.run_bass_kernel_spmd(
        nc, in_maps, core_ids=list(range(NCORES)), trace=trace
    )
    outT = np.concatenate([res.results[c]["outT"] for c in range(NCORES)], axis=1)
    h = np.ascontiguousarray(outT.T)
    if trace:
        return (h,), res
    return (h,)

